# revision 13
# baseline (speedup 1.0000x reference)
"""ConvBERT attention block (SeparableConv1D key + dynamic conv) on 8 TRN2 NeuronCores.

Sharding: data-parallel over batch (B=8 -> 1 sample per core), weights replicated.

Per-core dataflow ([C, L] layout, channels on partitions), software-pipelined over
four 512-column l-chunks so PE / ACT / DVE / Pool / DMA overlap across chunks:

  stage A (chunk a):  q = Wq^T x   (fp8 DoubleRow, 2x PE)
                      co = Wco^T x (bf16)
                      dwout = depthwise-conv x (plain-fp8 diag matmuls on PE)
                      key = pw^T dwout (fp8 DoubleRow)
                      attn = key * q (DVE)
  stage B (chunk a-1): logits = Wck^T attn (bf16) -> exp on ACT -> sums (ones
                      matmul) -> recip (DVE) -> 9-fold recip bcast via DRAM ->
                      expT normalized, staged to DRAM -> kx 64-way bcast DMAs
  stage C (chunk a-2): einsum out[c,l] = sum_k co[c,l+k-4]*kx[hk,l]
                      (windowed mult + tree adds, split DVE / Pool) -> out DMA

fp8 only feeds the q/key/logits path; its error is crushed by the softmax
(logits are tiny), leaving final rel-err ~5e-3. co stays bf16.
"""

import os
import sys

for _p in ("/opt/trn_rl_repo", "/root/.axon_site/_ro/trn_rl_repo"):
    if os.path.isdir(_p) and _p not in sys.path:
        sys.path.append(_p)

import ml_dtypes
import numpy as np

import concourse.bass as bass
import concourse.mybir as mybir
import concourse.tile as tile
from concourse import bacc
from concourse.bass_utils import run_bass_kernel_spmd

BF16 = mybir.dt.bfloat16
F32 = mybir.dt.float32
FP8 = mybir.dt.float8e4

H, D, K = 12, 64, 9
C = H * D  # 768
L = 2048
B = 8
PAD = (K - 1) // 2  # 4
P = 128
NCT = C // P  # 6 channel tiles
LC = 512
NLC = L // LC  # 4
HK = H * K  # 108
XROW = L + 16  # x8 row pitch: left pad 4, right pad 12 (keeps DR plane stride %16==0)
CROW = L + 8  # co row pitch (pad 4 both sides)

SW = 64.0  # fp8 weight scale (Wq, pw, dw)
SD = 32.0  # fp8 dwout scale

AF = mybir.ActivationFunctionType
OP = mybir.AluOpType
DR = mybir.MatmulPerfMode.DoubleRow

# einsum units routed to the Pool (gpsimd) engine instead of DVE
POOL_UNITS = {(0, 0), (0, 1), (0, 2), (0, 3), (1, 1)}




def _emit(nc, tc):
    from contextlib import ExitStack

    with ExitStack() as ctx:
        prs = ctx.enter_context(tc.tile_pool(name="prs", bufs=1))
        wcop = ctx.enter_context(tc.tile_pool(name="wcop", bufs=NCT))
        wckp = ctx.enter_context(tc.tile_pool(name="wckp", bufs=NCT))
        cop = ctx.enter_context(tc.tile_pool(name="cop", bufs=NCT))
        xtp = ctx.enter_context(tc.tile_pool(name="xtp", bufs=2))
        qp = ctx.enter_context(tc.tile_pool(name="qp", bufs=3))
        kp = ctx.enter_context(tc.tile_pool(name="kp", bufs=2))
        r9p = ctx.enter_context(tc.tile_pool(name="r9p", bufs=2))
        kxp = ctx.enter_context(tc.tile_pool(name="kxp", bufs=5))
        outp = ctx.enter_context(tc.tile_pool(name="outp", bufs=2))
        psb = ctx.enter_context(tc.tile_pool(name="psb", bufs=6, space="PSUM"))
        psl = ctx.enter_context(tc.tile_pool(name="psl", bufs=1, space="PSUM"))
        pss = ctx.enter_context(tc.tile_pool(name="pss", bufs=1, space="PSUM"))

        xT_d = nc.dram_tensor("xT", [C, L], BF16, kind="ExternalInput")
        x8_d = nc.dram_tensor("x8", [C, L], FP8, kind="ExternalInput")
        wq8_d = nc.dram_tensor("wq8", [P, 6 * C], FP8, kind="ExternalInput")
        pw8_d = nc.dram_tensor("pw8", [P, 6 * C], FP8, kind="ExternalInput")
        dg8_d = nc.dram_tensor("dg8", [P, NCT * K * P], FP8, kind="ExternalInput")
        wco_d = nc.dram_tensor("wco", [C, C], BF16, kind="ExternalInput")
        wck_d = nc.dram_tensor("wck", [C, HK], BF16, kind="ExternalInput")
        bq_d = nc.dram_tensor("bq", [P, NCT], F32, kind="ExternalInput")
        bco_d = nc.dram_tensor("bco", [P, NCT], F32, kind="ExternalInput")
        bsep_d = nc.dram_tensor("bsep", [P, NCT], F32, kind="ExternalInput")
        bck_d = nc.dram_tensor("bck", [HK, 1], F32, kind="ExternalInput")
        out_d = nc.dram_tensor("out", [C, L], BF16, kind="ExternalOutput")
        expT_dram = nc.dram_tensor("expTd", [HK, L], BF16)
        recipT_dram = nc.dram_tensor("recipTd", [H, L], BF16)

        # ---- persistent weights / constants ----
        wq8 = prs.tile([P, 6 * C], FP8, tag="wq8", name="wq8")
        pw8 = prs.tile([P, 6 * C], FP8, tag="pw8", name="pw8")
        dg8 = prs.tile([P, NCT * K * P], FP8, tag="dg8", name="dg8")
        nc.sync.dma_start(wq8[:], wq8_d[:])
        nc.sync.dma_start(pw8[:], pw8_d[:])
        nc.sync.dma_start(dg8[:], dg8_d[:])
        wco = [wcop.tile([P, C], BF16, tag="wco", name=f"wco{g}") for g in range(NCT)]
        wck = [wckp.tile([P, HK], BF16, tag="wck", name=f"wck{g}") for g in range(NCT)]
        for g in range(NCT):
            sl = slice(g * P, (g + 1) * P)
            nc.sync.dma_start(wco[g][:], wco_d[sl, :])
            nc.sync.dma_start(wck[g][:], wck_d[sl, :])
        bq = prs.tile([P, NCT], F32, tag="bq", name="bq")
        bco = prs.tile([P, NCT], F32, tag="bco", name="bco")
        bsep = prs.tile([P, NCT], F32, tag="bsep", name="bsep")
        bck = prs.tile([HK, 1], F32, tag="bck", name="bck")
        nc.sync.dma_start(bq[:], bq_d[:])
        nc.sync.dma_start(bco[:], bco_d[:])
        nc.sync.dma_start(bsep[:], bsep_d[:])
        nc.sync.dma_start(bck[:], bck_d[:])
        # ones_block[hk, h] = 1 iff hk // 9 == h (sums exp over k)
        ones = prs.tile([HK, H], BF16, tag="ones", name="ones")
        nc.gpsimd.memset(ones[:], 1.0)
        nc.gpsimd.affine_select(
            out=ones[:], in_=ones[:], compare_op=OP.is_ge, fill=0.0,
            base=0, pattern=[[-K, H]], channel_multiplier=1)
        nc.gpsimd.affine_select(
            out=ones[:], in_=ones[:], compare_op=OP.is_ge, fill=0.0,
            base=K - 1, pattern=[[K, H]], channel_multiplier=-1)

        # ---- persistent activations ----
        # x8: per ctile TWO fp8 rows: row 2g = x[l-4], row 2g+1 = x[l-3]
        # (duplicated shifted copy so conv tap pairs are DoubleRow-able with
        # plane stride XROW %16==0)
        x8 = prs.tile([P, 2 * NCT * XROW], FP8, tag="x8", name="x8")
        dw8 = prs.tile([P, NCT * L], FP8, tag="dw8", name="dw8")  # dwout * SD
        co = [cop.tile([P, CROW], BF16, tag="co", name=f"co{g}") for g in range(NCT)]
        expT = prs.tile([HK, L], BF16, tag="expT", name="expT")
        recipT = prs.tile([H, L], BF16, tag="recipT", name="recipT")
        for r in range(2 * NCT):
            nc.gpsimd.memset(
                bass.AP(x8.tensor, x8.offset + r * XROW,
                        [[2 * NCT * XROW, P], [1, PAD]]), 0.0)
            tail = PAD + L - (r % 2)
            nc.gpsimd.memset(
                bass.AP(x8.tensor, x8.offset + r * XROW + tail,
                        [[2 * NCT * XROW, P], [1, XROW - tail]]), 0.0)
        for g in range(NCT):
            nc.gpsimd.memset(co[g][:, 0:PAD], 0.0)
            nc.gpsimd.memset(co[g][:, PAD + L:CROW], 0.0)

        xap = [2 * NCT * XROW, P]

        def x8ap(row, off, dims):
            return bass.AP(x8.tensor, x8.offset + row * XROW + off,
                           [list(xap)] + dims)

        live = {}

        # ================= pipeline =================
        # emission order inside a step: loads, stage B (softmax+kx), stage C
        # (einsum), stage A (projections) — so each engine queue serves older
        # chunks first and the B/C chains never sit behind fresh A work.
        for s in range(NLC + 3):
            # ---- x loads for chunk s ----
            if s < NLC:
                cs = slice(s * LC, (s + 1) * LC)
                xt = xtp.tile([P, NCT * LC], BF16, tag="xt", name=f"xt{s}")
                nc.sync.dma_start(
                    bass.AP(xt.tensor, xt.offset, [[NCT * LC, P], [LC, NCT], [1, LC]]),
                    bass.AP(xT_d[:].tensor, s * LC, [[L, P], [P * L, NCT], [1, LC]]))
                live[("x", s)] = xt
                nc.sync.dma_start(
                    x8ap(0, PAD + s * LC, [[2 * XROW, NCT], [1, LC]]),
                    bass.AP(x8_d[:].tensor, s * LC, [[L, P], [P * L, NCT], [1, LC]]))
                w = LC if s < NLC - 1 else LC - 1
                nc.sync.dma_start(
                    x8ap(1, PAD + s * LC, [[2 * XROW, NCT], [1, w]]),
                    bass.AP(x8_d[:].tensor, s * LC + 1, [[L, P], [P * L, NCT], [1, w]]))

            # ---- stage B: softmax + kx broadcast for chunk b ----
            b = s - 2
            if 0 <= b < NLC:
                bsl = slice(b * LC, (b + 1) * LC)
                at = live[("q", b)]
                ps = psl.tile([HK, LC], F32, tag="psl", name="pslg")
                for g in range(NCT):
                    nc.tensor.matmul(ps[:], wck[g][:], at[:, g * LC:(g + 1) * LC],
                                     start=(g == 0), stop=(g == NCT - 1))
                nc.scalar.activation(expT[:, bsl], ps[:], AF.Exp, bias=bck[:, 0:1])
                ps2 = pss.tile([H, LC], F32, tag="pss", name="psss")
                nc.tensor.matmul(ps2[:], ones[:], expT[:, bsl], start=True, stop=True)
                with nc.allow_low_precision(reason="bf16 softmax denominators"):
                    nc.vector.reciprocal(recipT[:, bsl], ps2[:])
                nc.scalar.dma_start(recipT_dram[:, bsl], recipT[:, bsl])
                r9 = r9p.tile([HK, LC], BF16, tag="r9", name=f"r9_{b}")
                rb = recipT_dram[:]
                nc.scalar.dma_start(
                    r9[:], bass.AP(rb.tensor, b * LC, [[L, H], [0, K], [1, LC]]))
                nc.gpsimd.tensor_mul(expT[:, bsl], expT[:, bsl], r9[:])
                nc.scalar.dma_start(expT_dram[:, bsl], expT[:, bsl])
                eb = expT_dram[:]
                for g in range(NCT):
                    kx = kxp.tile([P, K, LC], BF16, tag="kx", name=f"kx{g}_{b}")
                    for hh in range(2):
                        sap = bass.AP(eb.tensor, K * (2 * g + hh) * L + b * LC,
                                      [[0, 64], [L, K], [1, LC]])
                        nc.sync.dma_start(kx[hh * 64:(hh + 1) * 64, :, :], sap)
                    live[("kx", g, b)] = kx

            # ---- stage C: dynamic-conv einsum for chunk c ----
            c = s - 3
            if 0 <= c < NLC:
                ot = outp.tile([P, NCT * LC], BF16, tag="o", name=f"o{c}")
                for g in range(NCT):
                    kx = live.pop(("kx", g, c))
                    base = co[g][:]
                    win = bass.AP(base.tensor, base.offset + c * LC,
                                  [list(base.ap)[0], [1, K], [1, LC]])
                    e = nc.gpsimd if (g, c) in POOL_UNITS else nc.vector
                    e.tensor_mul(kx[:], win, kx[:])
                    e.tensor_add(kx[:, 0:4, :], kx[:, 0:4, :], kx[:, 4:8, :])
                    e.tensor_add(kx[:, 0:2, :], kx[:, 0:2, :], kx[:, 2:4, :])
                    e.tensor_add(kx[:, 0, :], kx[:, 0, :], kx[:, 1, :])
                    e.tensor_add(ot[:, g * LC:(g + 1) * LC], kx[:, 0, :], kx[:, 8, :])
                nc.sync.dma_start(
                    bass.AP(out_d[:].tensor, c * LC, [[L, P], [P * L, NCT], [1, LC]]),
                    bass.AP(ot.tensor, ot.offset, [[NCT * LC, P], [LC, NCT], [1, LC]]))

            # ---- stage A: projections + conv + attn for chunk a ----
            a = s - 1
            if 0 <= a < NLC:
                xt = live[("x", a)]
                qt = qp.tile([P, NCT * LC], BF16, tag="q", name=f"q{a}")
                kt = kp.tile([P, NCT * LC], BF16, tag="k", name=f"k{a}")
                live[("q", a)] = qt
                # q (fp8 DoubleRow)
                for ot_ in range(NCT):
                    ps = psb.tile([P, LC], F32, tag="ps", name="psq")
                    for j in range(3):
                        lhsT = bass.AP(wq8.tensor, wq8.offset + j * 2 * C + ot_ * P,
                                       [list(wq8.ap[0]), [C, 2], [1, P]])
                        rhs = x8ap(4 * j, PAD + a * LC, [[2 * XROW, 2], [1, LC]])
                        nc.tensor.matmul(ps[:], lhsT, rhs, start=(j == 0),
                                         stop=(j == 2), perf_mode=DR)
                    nc.scalar.activation(qt[:, ot_ * LC:(ot_ + 1) * LC], ps[:],
                                         AF.Identity, bias=bq[:, ot_:ot_ + 1],
                                         scale=1.0 / SW)
                # co (bf16)
                for ot_ in range(NCT):
                    ps = psb.tile([P, LC], F32, tag="ps", name="psc")
                    for g in range(NCT):
                        nc.tensor.matmul(
                            ps[:], wco[g][:, ot_ * P:(ot_ + 1) * P],
                            xt[:, g * LC:(g + 1) * LC],
                            start=(g == 0), stop=(g == NCT - 1))
                    nc.scalar.activation(co[ot_][:, PAD + a * LC:PAD + (a + 1) * LC],
                                         ps[:], AF.Identity, bias=bco[:, ot_:ot_ + 1])
                # depthwise conv: 4 fp8 DoubleRow tap-pairs + plain tap 8
                for g in range(NCT):
                    ps = psb.tile([P, LC], F32, tag="ps", name="psd")
                    for j in range(4):
                        lhsT = bass.AP(dg8.tensor, dg8.offset + (g * K + 2 * j) * P,
                                       [list(dg8.ap[0]), [P, 2], [1, P]])
                        rhs = x8ap(2 * g, a * LC + 2 * j, [[XROW, 2], [1, LC]])
                        nc.tensor.matmul(ps[:], lhsT, rhs, start=(j == 0),
                                         stop=False, perf_mode=DR)
                    lhsT = bass.AP(dg8.tensor, dg8.offset + (g * K + 8) * P,
                                   [list(dg8.ap[0]), [1, P]])
                    rhs = x8ap(2 * g, a * LC + 8, [[1, LC]])
                    nc.tensor.matmul(ps[:], lhsT, rhs, start=False, stop=True)
                    nc.scalar.activation(
                        bass.AP(dw8.tensor, dw8.offset + g * L + a * LC,
                                [list(dw8.ap[0]), [1, LC]]),
                        ps[:], AF.Copy, scale=SD / SW)
                # key (fp8 DoubleRow) + attn mult
                for ot_ in range(NCT):
                    ps = psb.tile([P, LC], F32, tag="ps", name="psk")
                    for j in range(3):
                        lhsT = bass.AP(pw8.tensor, pw8.offset + j * 2 * C + ot_ * P,
                                       [list(pw8.ap[0]), [C, 2], [1, P]])
                        rhs = bass.AP(dw8.tensor, dw8.offset + 2 * j * L + a * LC,
                                      [list(dw8.ap[0]), [L, 2], [1, LC]])
                        nc.tensor.matmul(ps[:], lhsT, rhs, start=(j == 0),
                                         stop=(j == 2), perf_mode=DR)
                    nc.scalar.activation(kt[:, ot_ * LC:(ot_ + 1) * LC], ps[:],
                                         AF.Identity, bias=bsep[:, ot_:ot_ + 1],
                                         scale=1.0 / (SW * SD))
                nc.vector.tensor_mul(qt[:], qt[:], kt[:])


_NC_CACHE = None


def _build():
    global _NC_CACHE
    if _NC_CACHE is None:
        nc = bacc.Bacc("TRN2", target_bir_lowering=False, debug=False)
        with tile.TileContext(nc) as tc:
            _emit(nc, tc)
        nc.compile()
        _NC_CACHE = nc
    return _NC_CACHE


def _host_inputs(hidden_states, W_q, dw, pw, W_ck, W_co, b_q, b_co, sep_bias, b_ck):
    bf = ml_dtypes.bfloat16
    f8 = ml_dtypes.float8_e4m3
    wq8 = np.ascontiguousarray(
        (W_q * SW).reshape(3, 2, P, C).transpose(2, 0, 1, 3).reshape(P, 6 * C)
    ).astype(f8)
    pw8 = np.ascontiguousarray(
        (pw.T * SW).reshape(3, 2, P, C).transpose(2, 0, 1, 3).reshape(P, 6 * C)
    ).astype(f8)
    dws = np.asarray(dw, np.float32).reshape(C, K)
    dg = np.zeros((P, NCT, K, P), np.float32)
    idx = np.arange(P)
    for g in range(NCT):
        for k in range(K):
            dg[idx, g, k, idx] = dws[g * P + idx, k] * SW
    dg8 = np.ascontiguousarray(dg.reshape(P, NCT * K * P)).astype(f8)
    wco = W_co.astype(bf)
    wck = W_ck.astype(bf)
    bq = np.ascontiguousarray(b_q.reshape(NCT, P).T).astype(np.float32)
    bcoh = np.ascontiguousarray(b_co.reshape(NCT, P).T).astype(np.float32)
    bsep = np.ascontiguousarray(sep_bias.reshape(NCT, P).T).astype(np.float32)
    bck = np.asarray(b_ck, np.float32).reshape(HK, 1)
    shared = {"wq8": wq8, "pw8": pw8, "dg8": dg8, "wco": wco, "wck": wck,
              "bq": bq, "bco": bcoh, "bsep": bsep, "bck": bck}
    maps = []
    for b in range(B):
        xT = np.ascontiguousarray(np.asarray(hidden_states[b]).T)
        m = dict(shared)
        m["xT"] = xT.astype(bf)
        m["x8"] = xT.astype(f8)
        maps.append(m)
    return maps


def kernel(hidden_states, W_q, b_q, dw, pw, sep_bias, W_ck, b_ck, W_co, b_co):
    hidden_states = np.asarray(hidden_states, np.float32)
    nc = _build()
    maps = _host_inputs(hidden_states, np.asarray(W_q, np.float32),
                        np.asarray(dw, np.float32), np.asarray(pw, np.float32),
                        np.asarray(W_ck, np.float32), np.asarray(W_co, np.float32),
                        np.asarray(b_q, np.float32), np.asarray(b_co, np.float32),
                        np.asarray(sep_bias, np.float32), np.asarray(b_ck, np.float32))
    res = run_bass_kernel_spmd(nc, maps, list(range(B)))
    out = np.empty((B, L, C), np.float32)
    for b in range(B):
        out[b] = np.asarray(res.results[b]["out"]).T.astype(np.float32)
    return out


# revision 15
# speedup vs baseline: 1.0617x; 1.0617x over previous
"""ConvBERT attention block (SeparableConv1D key + dynamic conv) on 8 TRN2 NeuronCores.

Sharding: data-parallel over batch (B=8 -> 1 sample per core), weights replicated.

Per-core dataflow ([C, L] layout, channels on partitions), software-pipelined over
four 512-column l-chunks so PE / ACT / DVE / Pool / DMA overlap across chunks:

  stage A (chunk a):  q = Wq^T x   (fp8 DoubleRow, 2x PE)
                      co = Wco^T x (bf16)
                      dwout = depthwise-conv x (plain-fp8 diag matmuls on PE)
                      key = pw^T dwout (fp8 DoubleRow)
                      attn = key * q (DVE)
  stage B (chunk a-1): logits = Wck^T attn (bf16) -> exp on ACT -> sums (ones
                      matmul) -> recip (DVE) -> 9-fold recip bcast via DRAM ->
                      expT normalized, staged to DRAM -> kx 64-way bcast DMAs
  stage C (chunk a-2): einsum out[c,l] = sum_k co[c,l+k-4]*kx[hk,l]
                      (windowed mult + tree adds, split DVE / Pool) -> out DMA

fp8 only feeds the q/key/logits path; its error is crushed by the softmax
(logits are tiny), leaving final rel-err ~5e-3. co stays bf16.
"""

import os
import sys

for _p in ("/opt/trn_rl_repo", "/root/.axon_site/_ro/trn_rl_repo"):
    if os.path.isdir(_p) and _p not in sys.path:
        sys.path.append(_p)

import ml_dtypes
import numpy as np

import concourse.bass as bass
import concourse.mybir as mybir
import concourse.tile as tile
from concourse import bacc
from concourse.bass_utils import run_bass_kernel_spmd

BF16 = mybir.dt.bfloat16
F32 = mybir.dt.float32
FP8 = mybir.dt.float8e4

H, D, K = 12, 64, 9
C = H * D  # 768
L = 2048
B = 8
PAD = (K - 1) // 2  # 4
P = 128
NCT = C // P  # 6 channel tiles
LC = 256
NLC = L // LC  # 4
HK = H * K  # 108
XROW = L + 16  # x8 row pitch: left pad 4, right pad 12 (keeps DR plane stride %16==0)
CROW = L + 8  # co row pitch (pad 4 both sides)

SW = 64.0  # fp8 weight scale (Wq, pw, dw)
SD = 32.0  # fp8 dwout scale

AF = mybir.ActivationFunctionType
OP = mybir.AluOpType
DR = mybir.MatmulPerfMode.DoubleRow

# einsum units routed to the Pool (gpsimd) engine instead of DVE
POOL_UNITS = {(0, c) for c in range(8)} | {(1, 1), (1, 5)}




def _emit(nc, tc):
    from contextlib import ExitStack

    with ExitStack() as ctx:
        prs = ctx.enter_context(tc.tile_pool(name="prs", bufs=1))
        wcop = ctx.enter_context(tc.tile_pool(name="wcop", bufs=NCT))
        wckp = ctx.enter_context(tc.tile_pool(name="wckp", bufs=NCT))
        cop = ctx.enter_context(tc.tile_pool(name="cop", bufs=NCT))
        xtp = ctx.enter_context(tc.tile_pool(name="xtp", bufs=2))
        qp = ctx.enter_context(tc.tile_pool(name="qp", bufs=3))
        kp = ctx.enter_context(tc.tile_pool(name="kp", bufs=2))
        r9p = ctx.enter_context(tc.tile_pool(name="r9p", bufs=2))
        kxp = ctx.enter_context(tc.tile_pool(name="kxp", bufs=10))
        outp = ctx.enter_context(tc.tile_pool(name="outp", bufs=2))
        psb = ctx.enter_context(tc.tile_pool(name="psb", bufs=6, space="PSUM"))
        psl = ctx.enter_context(tc.tile_pool(name="psl", bufs=1, space="PSUM"))
        pss = ctx.enter_context(tc.tile_pool(name="pss", bufs=1, space="PSUM"))

        xT_d = nc.dram_tensor("xT", [C, L], BF16, kind="ExternalInput")
        x8_d = nc.dram_tensor("x8", [C, L], FP8, kind="ExternalInput")
        wq8_d = nc.dram_tensor("wq8", [P, 6 * C], FP8, kind="ExternalInput")
        pw8_d = nc.dram_tensor("pw8", [P, 6 * C], FP8, kind="ExternalInput")
        dg8_d = nc.dram_tensor("dg8", [P, NCT * K * P], FP8, kind="ExternalInput")
        wco_d = nc.dram_tensor("wco", [C, C], BF16, kind="ExternalInput")
        wck_d = nc.dram_tensor("wck", [C, HK], BF16, kind="ExternalInput")
        bq_d = nc.dram_tensor("bq", [P, NCT], F32, kind="ExternalInput")
        bco_d = nc.dram_tensor("bco", [P, NCT], F32, kind="ExternalInput")
        bsep_d = nc.dram_tensor("bsep", [P, NCT], F32, kind="ExternalInput")
        bck_d = nc.dram_tensor("bck", [HK, 1], F32, kind="ExternalInput")
        out_d = nc.dram_tensor("out", [C, L], BF16, kind="ExternalOutput")
        expT_dram = nc.dram_tensor("expTd", [HK, L], BF16)
        recipT_dram = nc.dram_tensor("recipTd", [H, L], BF16)

        # ---- persistent weights / constants ----
        wq8 = prs.tile([P, 6 * C], FP8, tag="wq8", name="wq8")
        pw8 = prs.tile([P, 6 * C], FP8, tag="pw8", name="pw8")
        dg8 = prs.tile([P, NCT * K * P], FP8, tag="dg8", name="dg8")
        nc.sync.dma_start(wq8[:], wq8_d[:])
        nc.sync.dma_start(pw8[:], pw8_d[:])
        nc.sync.dma_start(dg8[:], dg8_d[:])
        wco = [wcop.tile([P, C], BF16, tag="wco", name=f"wco{g}") for g in range(NCT)]
        wck = [wckp.tile([P, HK], BF16, tag="wck", name=f"wck{g}") for g in range(NCT)]
        for g in range(NCT):
            sl = slice(g * P, (g + 1) * P)
            nc.sync.dma_start(wco[g][:], wco_d[sl, :])
            nc.sync.dma_start(wck[g][:], wck_d[sl, :])
        bq = prs.tile([P, NCT], F32, tag="bq", name="bq")
        bco = prs.tile([P, NCT], F32, tag="bco", name="bco")
        bsep = prs.tile([P, NCT], F32, tag="bsep", name="bsep")
        bck = prs.tile([HK, 1], F32, tag="bck", name="bck")
        nc.sync.dma_start(bq[:], bq_d[:])
        nc.sync.dma_start(bco[:], bco_d[:])
        nc.sync.dma_start(bsep[:], bsep_d[:])
        nc.sync.dma_start(bck[:], bck_d[:])
        # ones_block[hk, h] = 1 iff hk // 9 == h (sums exp over k)
        ones = prs.tile([HK, H], BF16, tag="ones", name="ones")
        nc.gpsimd.memset(ones[:], 1.0)
        nc.gpsimd.affine_select(
            out=ones[:], in_=ones[:], compare_op=OP.is_ge, fill=0.0,
            base=0, pattern=[[-K, H]], channel_multiplier=1)
        nc.gpsimd.affine_select(
            out=ones[:], in_=ones[:], compare_op=OP.is_ge, fill=0.0,
            base=K - 1, pattern=[[K, H]], channel_multiplier=-1)

        # ---- persistent activations ----
        # x8: per ctile TWO fp8 rows: row 2g = x[l-4], row 2g+1 = x[l-3]
        # (duplicated shifted copy so conv tap pairs are DoubleRow-able with
        # plane stride XROW %16==0)
        x8 = prs.tile([P, 2 * NCT * XROW], FP8, tag="x8", name="x8")
        dw8 = prs.tile([P, NCT * L], FP8, tag="dw8", name="dw8")  # dwout * SD
        co = [cop.tile([P, CROW], BF16, tag="co", name=f"co{g}") for g in range(NCT)]
        expT = prs.tile([HK, L], BF16, tag="expT", name="expT")
        recipT = prs.tile([H, L], BF16, tag="recipT", name="recipT")
        for r in range(2 * NCT):
            nc.gpsimd.memset(
                bass.AP(x8.tensor, x8.offset + r * XROW,
                        [[2 * NCT * XROW, P], [1, PAD]]), 0.0)
            tail = PAD + L - (r % 2)
            nc.gpsimd.memset(
                bass.AP(x8.tensor, x8.offset + r * XROW + tail,
                        [[2 * NCT * XROW, P], [1, XROW - tail]]), 0.0)
        for g in range(NCT):
            nc.gpsimd.memset(co[g][:, 0:PAD], 0.0)
            nc.gpsimd.memset(co[g][:, PAD + L:CROW], 0.0)

        xap = [2 * NCT * XROW, P]

        def x8ap(row, off, dims):
            return bass.AP(x8.tensor, x8.offset + row * XROW + off,
                           [list(xap)] + dims)

        live = {}

        # ================= pipeline =================
        # emission order inside a step: loads, stage B (softmax+kx), stage C
        # (einsum), stage A (projections) — so each engine queue serves older
        # chunks first and the B/C chains never sit behind fresh A work.
        for s in range(NLC + 3):
            # ---- x loads for chunk s ----
            if s < NLC:
                cs = slice(s * LC, (s + 1) * LC)
                xt = xtp.tile([P, NCT * LC], BF16, tag="xt", name=f"xt{s}")
                nc.sync.dma_start(
                    bass.AP(xt.tensor, xt.offset, [[NCT * LC, P], [LC, NCT], [1, LC]]),
                    bass.AP(xT_d[:].tensor, s * LC, [[L, P], [P * L, NCT], [1, LC]]))
                live[("x", s)] = xt
                nc.sync.dma_start(
                    x8ap(0, PAD + s * LC, [[2 * XROW, NCT], [1, LC]]),
                    bass.AP(x8_d[:].tensor, s * LC, [[L, P], [P * L, NCT], [1, LC]]))
                w = LC if s < NLC - 1 else LC - 1
                nc.sync.dma_start(
                    x8ap(1, PAD + s * LC, [[2 * XROW, NCT], [1, w]]),
                    bass.AP(x8_d[:].tensor, s * LC + 1, [[L, P], [P * L, NCT], [1, w]]))

            # ---- stage B: softmax + kx broadcast for chunk b ----
            b = s - 2
            if 0 <= b < NLC:
                bsl = slice(b * LC, (b + 1) * LC)
                at = live[("q", b)]
                ps = psl.tile([HK, LC], F32, tag="psl", name="pslg")
                for g in range(NCT):
                    nc.tensor.matmul(ps[:], wck[g][:], at[:, g * LC:(g + 1) * LC],
                                     start=(g == 0), stop=(g == NCT - 1))
                nc.scalar.activation(expT[:, bsl], ps[:], AF.Exp, bias=bck[:, 0:1])
                ps2 = pss.tile([H, LC], F32, tag="pss", name="psss")
                nc.tensor.matmul(ps2[:], ones[:], expT[:, bsl], start=True, stop=True)
                with nc.allow_low_precision(reason="bf16 softmax denominators"):
                    nc.vector.reciprocal(recipT[:, bsl], ps2[:])
                nc.scalar.dma_start(recipT_dram[:, bsl], recipT[:, bsl])
                r9 = r9p.tile([HK, LC], BF16, tag="r9", name=f"r9_{b}")
                rb = recipT_dram[:]
                nc.scalar.dma_start(
                    r9[:], bass.AP(rb.tensor, b * LC, [[L, H], [0, K], [1, LC]]))
                nc.gpsimd.tensor_mul(expT[:, bsl], expT[:, bsl], r9[:])
                nc.scalar.dma_start(expT_dram[:, bsl], expT[:, bsl])
                eb = expT_dram[:]
                for g in range(NCT):
                    kx = kxp.tile([P, K, LC], BF16, tag="kx", name=f"kx{g}_{b}")
                    for hh in range(2):
                        sap = bass.AP(eb.tensor, K * (2 * g + hh) * L + b * LC,
                                      [[0, 64], [L, K], [1, LC]])
                        nc.sync.dma_start(kx[hh * 64:(hh + 1) * 64, :, :], sap)
                    live[("kx", g, b)] = kx

            # ---- stage C: dynamic-conv einsum for chunk c ----
            c = s - 3
            if 0 <= c < NLC:
                ot = outp.tile([P, NCT * LC], BF16, tag="o", name=f"o{c}")
                for g in range(NCT):
                    kx = live.pop(("kx", g, c))
                    base = co[g][:]
                    win = bass.AP(base.tensor, base.offset + c * LC,
                                  [list(base.ap)[0], [1, K], [1, LC]])
                    e = nc.gpsimd if (g, c) in POOL_UNITS else nc.vector
                    e.tensor_mul(kx[:], win, kx[:])
                    e.tensor_add(kx[:, 0:4, :], kx[:, 0:4, :], kx[:, 4:8, :])
                    e.tensor_add(kx[:, 0:2, :], kx[:, 0:2, :], kx[:, 2:4, :])
                    e.tensor_add(kx[:, 0, :], kx[:, 0, :], kx[:, 1, :])
                    e.tensor_add(ot[:, g * LC:(g + 1) * LC], kx[:, 0, :], kx[:, 8, :])
                nc.sync.dma_start(
                    bass.AP(out_d[:].tensor, c * LC, [[L, P], [P * L, NCT], [1, LC]]),
                    bass.AP(ot.tensor, ot.offset, [[NCT * LC, P], [LC, NCT], [1, LC]]))

            # ---- stage A: projections + conv + attn for chunk a ----
            a = s - 1
            if 0 <= a < NLC:
                xt = live[("x", a)]
                qt = qp.tile([P, NCT * LC], BF16, tag="q", name=f"q{a}")
                kt = kp.tile([P, NCT * LC], BF16, tag="k", name=f"k{a}")
                live[("q", a)] = qt
                # q (fp8 DoubleRow)
                for ot_ in range(NCT):
                    ps = psb.tile([P, LC], F32, tag="ps", name="psq")
                    for j in range(3):
                        lhsT = bass.AP(wq8.tensor, wq8.offset + j * 2 * C + ot_ * P,
                                       [list(wq8.ap[0]), [C, 2], [1, P]])
                        rhs = x8ap(4 * j, PAD + a * LC, [[2 * XROW, 2], [1, LC]])
                        nc.tensor.matmul(ps[:], lhsT, rhs, start=(j == 0),
                                         stop=(j == 2), perf_mode=DR)
                    nc.scalar.activation(qt[:, ot_ * LC:(ot_ + 1) * LC], ps[:],
                                         AF.Identity, bias=bq[:, ot_:ot_ + 1],
                                         scale=1.0 / SW)
                # co (bf16)
                for ot_ in range(NCT):
                    ps = psb.tile([P, LC], F32, tag="ps", name="psc")
                    for g in range(NCT):
                        nc.tensor.matmul(
                            ps[:], wco[g][:, ot_ * P:(ot_ + 1) * P],
                            xt[:, g * LC:(g + 1) * LC],
                            start=(g == 0), stop=(g == NCT - 1))
                    nc.scalar.activation(co[ot_][:, PAD + a * LC:PAD + (a + 1) * LC],
                                         ps[:], AF.Identity, bias=bco[:, ot_:ot_ + 1])
                # depthwise conv: 4 fp8 DoubleRow tap-pairs + plain tap 8
                for g in range(NCT):
                    ps = psb.tile([P, LC], F32, tag="ps", name="psd")
                    for j in range(4):
                        lhsT = bass.AP(dg8.tensor, dg8.offset + (g * K + 2 * j) * P,
                                       [list(dg8.ap[0]), [P, 2], [1, P]])
                        rhs = x8ap(2 * g, a * LC + 2 * j, [[XROW, 2], [1, LC]])
                        nc.tensor.matmul(ps[:], lhsT, rhs, start=(j == 0),
                                         stop=False, perf_mode=DR)
                    lhsT = bass.AP(dg8.tensor, dg8.offset + (g * K + 8) * P,
                                   [list(dg8.ap[0]), [1, P]])
                    rhs = x8ap(2 * g, a * LC + 8, [[1, LC]])
                    nc.tensor.matmul(ps[:], lhsT, rhs, start=False, stop=True)
                    nc.scalar.activation(
                        bass.AP(dw8.tensor, dw8.offset + g * L + a * LC,
                                [list(dw8.ap[0]), [1, LC]]),
                        ps[:], AF.Copy, scale=SD / SW)
                # key (fp8 DoubleRow) + attn mult
                for ot_ in range(NCT):
                    ps = psb.tile([P, LC], F32, tag="ps", name="psk")
                    for j in range(3):
                        lhsT = bass.AP(pw8.tensor, pw8.offset + j * 2 * C + ot_ * P,
                                       [list(pw8.ap[0]), [C, 2], [1, P]])
                        rhs = bass.AP(dw8.tensor, dw8.offset + 2 * j * L + a * LC,
                                      [list(dw8.ap[0]), [L, 2], [1, LC]])
                        nc.tensor.matmul(ps[:], lhsT, rhs, start=(j == 0),
                                         stop=(j == 2), perf_mode=DR)
                    nc.scalar.activation(kt[:, ot_ * LC:(ot_ + 1) * LC], ps[:],
                                         AF.Identity, bias=bsep[:, ot_:ot_ + 1],
                                         scale=1.0 / (SW * SD))
                nc.vector.tensor_mul(qt[:], qt[:], kt[:])


_NC_CACHE = None


def _build():
    global _NC_CACHE
    if _NC_CACHE is None:
        nc = bacc.Bacc("TRN2", target_bir_lowering=False, debug=False)
        with tile.TileContext(nc) as tc:
            _emit(nc, tc)
        nc.compile()
        _NC_CACHE = nc
    return _NC_CACHE


def _host_inputs(hidden_states, W_q, dw, pw, W_ck, W_co, b_q, b_co, sep_bias, b_ck):
    bf = ml_dtypes.bfloat16
    f8 = ml_dtypes.float8_e4m3
    wq8 = np.ascontiguousarray(
        (W_q * SW).reshape(3, 2, P, C).transpose(2, 0, 1, 3).reshape(P, 6 * C)
    ).astype(f8)
    pw8 = np.ascontiguousarray(
        (pw.T * SW).reshape(3, 2, P, C).transpose(2, 0, 1, 3).reshape(P, 6 * C)
    ).astype(f8)
    dws = np.asarray(dw, np.float32).reshape(C, K)
    dg = np.zeros((P, NCT, K, P), np.float32)
    idx = np.arange(P)
    for g in range(NCT):
        for k in range(K):
            dg[idx, g, k, idx] = dws[g * P + idx, k] * SW
    dg8 = np.ascontiguousarray(dg.reshape(P, NCT * K * P)).astype(f8)
    wco = W_co.astype(bf)
    wck = W_ck.astype(bf)
    bq = np.ascontiguousarray(b_q.reshape(NCT, P).T).astype(np.float32)
    bcoh = np.ascontiguousarray(b_co.reshape(NCT, P).T).astype(np.float32)
    bsep = np.ascontiguousarray(sep_bias.reshape(NCT, P).T).astype(np.float32)
    bck = np.asarray(b_ck, np.float32).reshape(HK, 1)
    shared = {"wq8": wq8, "pw8": pw8, "dg8": dg8, "wco": wco, "wck": wck,
              "bq": bq, "bco": bcoh, "bsep": bsep, "bck": bck}
    maps = []
    for b in range(B):
        xT = np.ascontiguousarray(np.asarray(hidden_states[b]).T)
        m = dict(shared)
        m["xT"] = xT.astype(bf)
        m["x8"] = xT.astype(f8)
        maps.append(m)
    return maps


def kernel(hidden_states, W_q, b_q, dw, pw, sep_bias, W_ck, b_ck, W_co, b_co):
    hidden_states = np.asarray(hidden_states, np.float32)
    nc = _build()
    maps = _host_inputs(hidden_states, np.asarray(W_q, np.float32),
                        np.asarray(dw, np.float32), np.asarray(pw, np.float32),
                        np.asarray(W_ck, np.float32), np.asarray(W_co, np.float32),
                        np.asarray(b_q, np.float32), np.asarray(b_co, np.float32),
                        np.asarray(sep_bias, np.float32), np.asarray(b_ck, np.float32))
    res = run_bass_kernel_spmd(nc, maps, list(range(B)))
    out = np.empty((B, L, C), np.float32)
    for b in range(B):
        out[b] = np.asarray(res.results[b]["out"]).T.astype(np.float32)
    return out


# revision 16
# speedup vs baseline: 1.0709x; 1.0087x over previous
"""ConvBERT attention block (SeparableConv1D key + dynamic conv) on 8 TRN2 NeuronCores.

Sharding: data-parallel over batch (B=8 -> 1 sample per core), weights replicated.

Per-core dataflow ([C, L] layout, channels on partitions), software-pipelined over
four 512-column l-chunks so PE / ACT / DVE / Pool / DMA overlap across chunks:

  stage A (chunk a):  q = Wq^T x   (fp8 DoubleRow, 2x PE)
                      co = Wco^T x (bf16)
                      dwout = depthwise-conv x (plain-fp8 diag matmuls on PE)
                      key = pw^T dwout (fp8 DoubleRow)
                      attn = key * q (DVE)
  stage B (chunk a-1): logits = Wck^T attn (bf16) -> exp on ACT -> sums (ones
                      matmul) -> recip (DVE) -> 9-fold recip bcast via DRAM ->
                      expT normalized, staged to DRAM -> kx 64-way bcast DMAs
  stage C (chunk a-2): einsum out[c,l] = sum_k co[c,l+k-4]*kx[hk,l]
                      (windowed mult + tree adds, split DVE / Pool) -> out DMA

fp8 only feeds the q/key/logits path; its error is crushed by the softmax
(logits are tiny), leaving final rel-err ~5e-3. co stays bf16.
"""

import os
import sys

for _p in ("/opt/trn_rl_repo", "/root/.axon_site/_ro/trn_rl_repo"):
    if os.path.isdir(_p) and _p not in sys.path:
        sys.path.append(_p)

import ml_dtypes
import numpy as np

import concourse.bass as bass
import concourse.mybir as mybir
import concourse.tile as tile
from concourse import bacc
from concourse.bass_utils import run_bass_kernel_spmd

BF16 = mybir.dt.bfloat16
F32 = mybir.dt.float32
FP8 = mybir.dt.float8e4

H, D, K = 12, 64, 9
C = H * D  # 768
L = 2048
B = 8
PAD = (K - 1) // 2  # 4
P = 128
NCT = C // P  # 6 channel tiles
LC = 256
NLC = L // LC  # 4
HK = H * K  # 108
XROW = L + 16  # x8 row pitch: left pad 4, right pad 12 (keeps DR plane stride %16==0)
CROW = L + 8  # co row pitch (pad 4 both sides)

SW = 64.0  # fp8 weight scale (Wq, pw, dw)
SD = 32.0  # fp8 dwout scale

AF = mybir.ActivationFunctionType
OP = mybir.AluOpType
DR = mybir.MatmulPerfMode.DoubleRow

# einsum units routed to the Pool (gpsimd) engine instead of DVE
POOL_UNITS = {(0, c) for c in range(8)} | {(1, 1), (1, 5)}




def _emit(nc, tc):
    from contextlib import ExitStack

    with ExitStack() as ctx:
        prs = ctx.enter_context(tc.tile_pool(name="prs", bufs=1))
        wcop = ctx.enter_context(tc.tile_pool(name="wcop", bufs=NCT))
        wckp = ctx.enter_context(tc.tile_pool(name="wckp", bufs=NCT))
        cop = ctx.enter_context(tc.tile_pool(name="cop", bufs=NCT))
        xtp = ctx.enter_context(tc.tile_pool(name="xtp", bufs=2))
        qp = ctx.enter_context(tc.tile_pool(name="qp", bufs=3))
        kp = ctx.enter_context(tc.tile_pool(name="kp", bufs=2))
        r9p = ctx.enter_context(tc.tile_pool(name="r9p", bufs=2))
        kxp = ctx.enter_context(tc.tile_pool(name="kxp", bufs=10))
        outp = ctx.enter_context(tc.tile_pool(name="outp", bufs=2))
        psb = ctx.enter_context(tc.tile_pool(name="psb", bufs=6, space="PSUM"))
        psl = ctx.enter_context(tc.tile_pool(name="psl", bufs=1, space="PSUM"))
        pss = ctx.enter_context(tc.tile_pool(name="pss", bufs=1, space="PSUM"))

        xT_d = nc.dram_tensor("xT", [C, L], BF16, kind="ExternalInput")
        x8_d = nc.dram_tensor("x8", [C, L], FP8, kind="ExternalInput")
        wq8_d = nc.dram_tensor("wq8", [P, 6 * C], FP8, kind="ExternalInput")
        pw8_d = nc.dram_tensor("pw8", [P, 6 * C], FP8, kind="ExternalInput")
        dg8_d = nc.dram_tensor("dg8", [P, NCT * K * P], FP8, kind="ExternalInput")
        wco_d = nc.dram_tensor("wco", [C, C], BF16, kind="ExternalInput")
        wck_d = nc.dram_tensor("wck", [C, HK], BF16, kind="ExternalInput")
        bq_d = nc.dram_tensor("bq", [P, NCT], F32, kind="ExternalInput")
        bco_d = nc.dram_tensor("bco", [P, NCT], F32, kind="ExternalInput")
        bsep_d = nc.dram_tensor("bsep", [P, NCT], F32, kind="ExternalInput")
        bck_d = nc.dram_tensor("bck", [HK, 1], F32, kind="ExternalInput")
        out_d = nc.dram_tensor("out", [C, L], BF16, kind="ExternalOutput")
        expT_dram = nc.dram_tensor("expTd", [HK, L], BF16)
        recipT_dram = nc.dram_tensor("recipTd", [H, L], BF16)

        # ---- persistent weights / constants ----
        wq8 = prs.tile([P, 6 * C], FP8, tag="wq8", name="wq8")
        pw8 = prs.tile([P, 6 * C], FP8, tag="pw8", name="pw8")
        dg8 = prs.tile([P, NCT * K * P], FP8, tag="dg8", name="dg8")
        nc.sync.dma_start(wq8[:], wq8_d[:])
        nc.sync.dma_start(pw8[:], pw8_d[:])
        nc.sync.dma_start(dg8[:], dg8_d[:])
        wco = [wcop.tile([P, C], BF16, tag="wco", name=f"wco{g}") for g in range(NCT)]
        wck = [wckp.tile([P, HK], BF16, tag="wck", name=f"wck{g}") for g in range(NCT)]
        for g in range(NCT):
            sl = slice(g * P, (g + 1) * P)
            nc.sync.dma_start(wco[g][:], wco_d[sl, :])
            nc.sync.dma_start(wck[g][:], wck_d[sl, :])
        bq = prs.tile([P, NCT], F32, tag="bq", name="bq")
        bco = prs.tile([P, NCT], F32, tag="bco", name="bco")
        bsep = prs.tile([P, NCT], F32, tag="bsep", name="bsep")
        bck = prs.tile([HK, 1], F32, tag="bck", name="bck")
        nc.sync.dma_start(bq[:], bq_d[:])
        nc.sync.dma_start(bco[:], bco_d[:])
        nc.sync.dma_start(bsep[:], bsep_d[:])
        nc.sync.dma_start(bck[:], bck_d[:])
        # ones_block[hk, h] = 1 iff hk // 9 == h (sums exp over k)
        ones = prs.tile([HK, H], BF16, tag="ones", name="ones")
        nc.gpsimd.memset(ones[:], 1.0)
        nc.gpsimd.affine_select(
            out=ones[:], in_=ones[:], compare_op=OP.is_ge, fill=0.0,
            base=0, pattern=[[-K, H]], channel_multiplier=1)
        nc.gpsimd.affine_select(
            out=ones[:], in_=ones[:], compare_op=OP.is_ge, fill=0.0,
            base=K - 1, pattern=[[K, H]], channel_multiplier=-1)

        # ---- persistent activations ----
        # x8: per ctile TWO fp8 rows: row 2g = x[l-4], row 2g+1 = x[l-3]
        # (duplicated shifted copy so conv tap pairs are DoubleRow-able with
        # plane stride XROW %16==0)
        x8 = prs.tile([P, 2 * NCT * XROW], FP8, tag="x8", name="x8")
        dw8 = prs.tile([P, NCT * L], FP8, tag="dw8", name="dw8")  # dwout * SD
        co = [cop.tile([P, CROW], BF16, tag="co", name=f"co{g}") for g in range(NCT)]
        expT = prs.tile([HK, L], BF16, tag="expT", name="expT")
        recipT = prs.tile([H, L], BF16, tag="recipT", name="recipT")
        for r in range(2 * NCT):
            nc.gpsimd.memset(
                bass.AP(x8.tensor, x8.offset + r * XROW,
                        [[2 * NCT * XROW, P], [1, PAD]]), 0.0)
            tail = PAD + L - (r % 2)
            nc.gpsimd.memset(
                bass.AP(x8.tensor, x8.offset + r * XROW + tail,
                        [[2 * NCT * XROW, P], [1, XROW - tail]]), 0.0)
        for g in range(NCT):
            nc.gpsimd.memset(co[g][:, 0:PAD], 0.0)
            nc.gpsimd.memset(co[g][:, PAD + L:CROW], 0.0)

        xap = [2 * NCT * XROW, P]

        def x8ap(row, off, dims):
            return bass.AP(x8.tensor, x8.offset + row * XROW + off,
                           [list(xap)] + dims)

        live = {}

        # ================= pipeline =================
        # Per-step emission order is tuned so no engine queue-head waits on
        # another engine's queue tail: logits/exp (b) first, 2 einsum units
        # (c), softmax tail + kx (b), conv/key/q (a), attn (a), remaining
        # einsum units (c), co (a) last (not needed for 2 more steps).
        for s in range(NLC + 3):
            if s < NLC:
                cs = slice(s * LC, (s + 1) * LC)
                xt = xtp.tile([P, NCT * LC], BF16, tag="xt", name=f"xt{s}")
                nc.sync.dma_start(
                    bass.AP(xt.tensor, xt.offset, [[NCT * LC, P], [LC, NCT], [1, LC]]),
                    bass.AP(xT_d[:].tensor, s * LC, [[L, P], [P * L, NCT], [1, LC]]))
                live[("x", s)] = xt
                nc.sync.dma_start(
                    x8ap(0, PAD + s * LC, [[2 * XROW, NCT], [1, LC]]),
                    bass.AP(x8_d[:].tensor, s * LC, [[L, P], [P * L, NCT], [1, LC]]))
                w = LC if s < NLC - 1 else LC - 1
                nc.sync.dma_start(
                    x8ap(1, PAD + s * LC, [[2 * XROW, NCT], [1, w]]),
                    bass.AP(x8_d[:].tensor, s * LC + 1, [[L, P], [P * L, NCT], [1, w]]))

            a, b, c = s - 1, s - 2, s - 3
            bsl = slice(b * LC, (b + 1) * LC)

            # B1: logits + exp
            if 0 <= b < NLC:
                at = live[("q", b)]
                ps_l = psl.tile([HK, LC], F32, tag="psl", name="pslg")
                for g in range(NCT):
                    nc.tensor.matmul(ps_l[:], wck[g][:], at[:, g * LC:(g + 1) * LC],
                                     start=(g == 0), stop=(g == NCT - 1))
                nc.scalar.activation(expT[:, bsl], ps_l[:], AF.Exp, bias=bck[:, 0:1])

            # C part 1: einsum units g=0,1
            def einsum_unit(g, c, ot):
                kx = live.pop(("kx", g, c))
                base = co[g][:]
                win = bass.AP(base.tensor, base.offset + c * LC,
                              [list(base.ap)[0], [1, K], [1, LC]])
                e = nc.gpsimd if (g, c) in POOL_UNITS else nc.vector
                e.tensor_mul(kx[:], win, kx[:])
                e.tensor_add(kx[:, 0:4, :], kx[:, 0:4, :], kx[:, 4:8, :])
                e.tensor_add(kx[:, 0:2, :], kx[:, 0:2, :], kx[:, 2:4, :])
                e.tensor_add(kx[:, 0, :], kx[:, 0, :], kx[:, 1, :])
                e.tensor_add(ot[:, g * LC:(g + 1) * LC], kx[:, 0, :], kx[:, 8, :])

            otile = None
            if 0 <= c < NLC:
                otile = outp.tile([P, NCT * LC], BF16, tag="o", name=f"o{c}")
                for g in (0, 1):
                    einsum_unit(g, c, otile)

            # B2: sums, recip, recip9 fold, staging, kx broadcast
            if 0 <= b < NLC:
                ps2 = pss.tile([H, LC], F32, tag="pss", name="psss")
                nc.tensor.matmul(ps2[:], ones[:], expT[:, bsl], start=True, stop=True)
                with nc.allow_low_precision(reason="bf16 softmax denominators"):
                    nc.vector.reciprocal(recipT[:, bsl], ps2[:])
                nc.scalar.dma_start(recipT_dram[:, bsl], recipT[:, bsl])
                r9 = r9p.tile([HK, LC], BF16, tag="r9", name=f"r9_{b}")
                rb = recipT_dram[:]
                nc.scalar.dma_start(
                    r9[:], bass.AP(rb.tensor, b * LC, [[L, H], [0, K], [1, LC]]))
                nc.gpsimd.tensor_mul(expT[:, bsl], expT[:, bsl], r9[:])
                nc.scalar.dma_start(expT_dram[:, bsl], expT[:, bsl])
                eb = expT_dram[:]
                for g in range(NCT):
                    kx = kxp.tile([P, K, LC], BF16, tag="kx", name=f"kx{g}_{b}")
                    for hh in range(2):
                        sap = bass.AP(eb.tensor, K * (2 * g + hh) * L + b * LC,
                                      [[0, 64], [L, K], [1, LC]])
                        nc.sync.dma_start(kx[hh * 64:(hh + 1) * 64, :, :], sap)
                    live[("kx", g, b)] = kx

            # A: conv -> key -> q (co deferred)
            if 0 <= a < NLC:
                xt = live[("x", a)]
                qt = qp.tile([P, NCT * LC], BF16, tag="q", name=f"q{a}")
                kt = kp.tile([P, NCT * LC], BF16, tag="k", name=f"k{a}")
                live[("q", a)] = qt
                for g in range(NCT):
                    ps = psb.tile([P, LC], F32, tag="ps", name="psd")
                    for j in range(4):
                        lhsT = bass.AP(dg8.tensor, dg8.offset + (g * K + 2 * j) * P,
                                       [list(dg8.ap[0]), [P, 2], [1, P]])
                        rhs = x8ap(2 * g, a * LC + 2 * j, [[XROW, 2], [1, LC]])
                        nc.tensor.matmul(ps[:], lhsT, rhs, start=(j == 0),
                                         stop=False, perf_mode=DR)
                    lhsT = bass.AP(dg8.tensor, dg8.offset + (g * K + 8) * P,
                                   [list(dg8.ap[0]), [1, P]])
                    rhs = x8ap(2 * g, a * LC + 8, [[1, LC]])
                    nc.tensor.matmul(ps[:], lhsT, rhs, start=False, stop=True)
                    nc.scalar.activation(
                        bass.AP(dw8.tensor, dw8.offset + g * L + a * LC,
                                [list(dw8.ap[0]), [1, LC]]),
                        ps[:], AF.Copy, scale=SD / SW)
                for ot_ in range(NCT):
                    ps = psb.tile([P, LC], F32, tag="ps", name="psk")
                    for j in range(3):
                        lhsT = bass.AP(pw8.tensor, pw8.offset + j * 2 * C + ot_ * P,
                                       [list(pw8.ap[0]), [C, 2], [1, P]])
                        rhs = bass.AP(dw8.tensor, dw8.offset + 2 * j * L + a * LC,
                                      [list(dw8.ap[0]), [L, 2], [1, LC]])
                        nc.tensor.matmul(ps[:], lhsT, rhs, start=(j == 0),
                                         stop=(j == 2), perf_mode=DR)
                    nc.scalar.activation(kt[:, ot_ * LC:(ot_ + 1) * LC], ps[:],
                                         AF.Identity, bias=bsep[:, ot_:ot_ + 1],
                                         scale=1.0 / (SW * SD))
                for ot_ in range(NCT):
                    ps = psb.tile([P, LC], F32, tag="ps", name="psq")
                    for j in range(3):
                        lhsT = bass.AP(wq8.tensor, wq8.offset + j * 2 * C + ot_ * P,
                                       [list(wq8.ap[0]), [C, 2], [1, P]])
                        rhs = x8ap(4 * j, PAD + a * LC, [[2 * XROW, 2], [1, LC]])
                        nc.tensor.matmul(ps[:], lhsT, rhs, start=(j == 0),
                                         stop=(j == 2), perf_mode=DR)
                    nc.scalar.activation(qt[:, ot_ * LC:(ot_ + 1) * LC], ps[:],
                                         AF.Identity, bias=bq[:, ot_:ot_ + 1],
                                         scale=1.0 / SW)
                # attn (per-ot so each fires as soon as its evacs land)
                for ot_ in range(NCT):
                    sl_ = slice(ot_ * LC, (ot_ + 1) * LC)
                    nc.vector.tensor_mul(qt[:, sl_], qt[:, sl_], kt[:, sl_])

            # C part 2 + out store
            if 0 <= c < NLC:
                for g in range(2, NCT):
                    einsum_unit(g, c, otile)
                nc.sync.dma_start(
                    bass.AP(out_d[:].tensor, c * LC, [[L, P], [P * L, NCT], [1, LC]]),
                    bass.AP(otile.tensor, otile.offset,
                            [[NCT * LC, P], [LC, NCT], [1, LC]]))

            # A tail: co (bf16) — consumed by einsum two steps later
            if 0 <= a < NLC:
                xt = live[("x", a)]
                for ot_ in range(NCT):
                    ps = psb.tile([P, LC], F32, tag="ps", name="psc")
                    for g in range(NCT):
                        nc.tensor.matmul(
                            ps[:], wco[g][:, ot_ * P:(ot_ + 1) * P],
                            xt[:, g * LC:(g + 1) * LC],
                            start=(g == 0), stop=(g == NCT - 1))
                    nc.scalar.activation(co[ot_][:, PAD + a * LC:PAD + (a + 1) * LC],
                                         ps[:], AF.Identity, bias=bco[:, ot_:ot_ + 1])


_NC_CACHE = None


def _build():
    global _NC_CACHE
    if _NC_CACHE is None:
        nc = bacc.Bacc("TRN2", target_bir_lowering=False, debug=False)
        with tile.TileContext(nc) as tc:
            _emit(nc, tc)
        nc.compile()
        _NC_CACHE = nc
    return _NC_CACHE


def _host_inputs(hidden_states, W_q, dw, pw, W_ck, W_co, b_q, b_co, sep_bias, b_ck):
    bf = ml_dtypes.bfloat16
    f8 = ml_dtypes.float8_e4m3
    wq8 = np.ascontiguousarray(
        (W_q * SW).reshape(3, 2, P, C).transpose(2, 0, 1, 3).reshape(P, 6 * C)
    ).astype(f8)
    pw8 = np.ascontiguousarray(
        (pw.T * SW).reshape(3, 2, P, C).transpose(2, 0, 1, 3).reshape(P, 6 * C)
    ).astype(f8)
    dws = np.asarray(dw, np.float32).reshape(C, K)
    dg = np.zeros((P, NCT, K, P), np.float32)
    idx = np.arange(P)
    for g in range(NCT):
        for k in range(K):
            dg[idx, g, k, idx] = dws[g * P + idx, k] * SW
    dg8 = np.ascontiguousarray(dg.reshape(P, NCT * K * P)).astype(f8)
    wco = W_co.astype(bf)
    wck = W_ck.astype(bf)
    bq = np.ascontiguousarray(b_q.reshape(NCT, P).T).astype(np.float32)
    bcoh = np.ascontiguousarray(b_co.reshape(NCT, P).T).astype(np.float32)
    bsep = np.ascontiguousarray(sep_bias.reshape(NCT, P).T).astype(np.float32)
    bck = np.asarray(b_ck, np.float32).reshape(HK, 1)
    shared = {"wq8": wq8, "pw8": pw8, "dg8": dg8, "wco": wco, "wck": wck,
              "bq": bq, "bco": bcoh, "bsep": bsep, "bck": bck}
    maps = []
    for b in range(B):
        xT = np.ascontiguousarray(np.asarray(hidden_states[b]).T)
        m = dict(shared)
        m["xT"] = xT.astype(bf)
        m["x8"] = xT.astype(f8)
        maps.append(m)
    return maps


def kernel(hidden_states, W_q, b_q, dw, pw, sep_bias, W_ck, b_ck, W_co, b_co):
    hidden_states = np.asarray(hidden_states, np.float32)
    nc = _build()
    maps = _host_inputs(hidden_states, np.asarray(W_q, np.float32),
                        np.asarray(dw, np.float32), np.asarray(pw, np.float32),
                        np.asarray(W_ck, np.float32), np.asarray(W_co, np.float32),
                        np.asarray(b_q, np.float32), np.asarray(b_co, np.float32),
                        np.asarray(sep_bias, np.float32), np.asarray(b_ck, np.float32))
    res = run_bass_kernel_spmd(nc, maps, list(range(B)))
    out = np.empty((B, L, C), np.float32)
    for b in range(B):
        out[b] = np.asarray(res.results[b]["out"]).T.astype(np.float32)
    return out


# revision 17
# speedup vs baseline: 1.1080x; 1.0346x over previous
"""ConvBERT attention block (SeparableConv1D key + dynamic conv) on 8 TRN2 NeuronCores.

Sharding: data-parallel over batch (B=8 -> 1 sample per core), weights replicated.

Per-core dataflow ([C, L] layout, channels on partitions), software-pipelined over
four 512-column l-chunks so PE / ACT / DVE / Pool / DMA overlap across chunks:

  stage A (chunk a):  q = Wq^T x   (fp8 DoubleRow, 2x PE)
                      co = Wco^T x (bf16)
                      dwout = depthwise-conv x (plain-fp8 diag matmuls on PE)
                      key = pw^T dwout (fp8 DoubleRow)
                      attn = key * q (DVE)
  stage B (chunk a-1): logits = Wck^T attn (bf16) -> exp on ACT -> sums (ones
                      matmul) -> recip (DVE) -> 9-fold recip bcast via DRAM ->
                      expT normalized, staged to DRAM -> kx 64-way bcast DMAs
  stage C (chunk a-2): einsum out[c,l] = sum_k co[c,l+k-4]*kx[hk,l]
                      (windowed mult + tree adds, split DVE / Pool) -> out DMA

fp8 only feeds the q/key/logits path; its error is crushed by the softmax
(logits are tiny), leaving final rel-err ~5e-3. co stays bf16.
"""

import os
import sys

for _p in ("/opt/trn_rl_repo", "/root/.axon_site/_ro/trn_rl_repo"):
    if os.path.isdir(_p) and _p not in sys.path:
        sys.path.append(_p)

import ml_dtypes
import numpy as np

import concourse.bass as bass
import concourse.mybir as mybir
import concourse.tile as tile
from concourse import bacc
from concourse.bass_utils import run_bass_kernel_spmd

BF16 = mybir.dt.bfloat16
F32 = mybir.dt.float32
FP8 = mybir.dt.float8e4

H, D, K = 12, 64, 9
C = H * D  # 768
L = 2048
B = 8
PAD = (K - 1) // 2  # 4
P = 128
NCT = C // P  # 6 channel tiles
LC = 256
NLC = L // LC  # 4
HK = H * K  # 108
XROW = L + 16  # x8 row pitch: left pad 4, right pad 12 (keeps DR plane stride %16==0)
CROW = L + 8  # co row pitch (pad 4 both sides)

SW = 64.0  # fp8 weight scale (Wq, pw, dw)
SD = 32.0  # fp8 dwout scale

AF = mybir.ActivationFunctionType
OP = mybir.AluOpType
DR = mybir.MatmulPerfMode.DoubleRow

# einsum units routed to the Pool (gpsimd) engine instead of DVE
POOL_UNITS = {(2, c) for c in range(8)} | {(3, 1), (3, 5)}




def _emit(nc, tc):
    from contextlib import ExitStack

    with ExitStack() as ctx:
        prs = ctx.enter_context(tc.tile_pool(name="prs", bufs=1))
        wcop = ctx.enter_context(tc.tile_pool(name="wcop", bufs=NCT))
        wckp = ctx.enter_context(tc.tile_pool(name="wckp", bufs=NCT))
        cop = ctx.enter_context(tc.tile_pool(name="cop", bufs=NCT))
        xtp = ctx.enter_context(tc.tile_pool(name="xtp", bufs=3))
        qp = ctx.enter_context(tc.tile_pool(name="qp", bufs=4))
        kp = ctx.enter_context(tc.tile_pool(name="kp", bufs=3))
        r9p = ctx.enter_context(tc.tile_pool(name="r9p", bufs=3))
        kxp = ctx.enter_context(tc.tile_pool(name="kxp", bufs=10))
        outp = ctx.enter_context(tc.tile_pool(name="outp", bufs=3))
        psb = ctx.enter_context(tc.tile_pool(name="psb", bufs=6, space="PSUM"))
        psl = ctx.enter_context(tc.tile_pool(name="psl", bufs=1, space="PSUM"))
        pss = ctx.enter_context(tc.tile_pool(name="pss", bufs=1, space="PSUM"))

        xT_d = nc.dram_tensor("xT", [C, L], BF16, kind="ExternalInput")
        x8_d = nc.dram_tensor("x8", [C, L], FP8, kind="ExternalInput")
        wq8_d = nc.dram_tensor("wq8", [P, 6 * C], FP8, kind="ExternalInput")
        pw8_d = nc.dram_tensor("pw8", [P, 6 * C], FP8, kind="ExternalInput")
        dg8_d = nc.dram_tensor("dg8", [P, NCT * K * P], FP8, kind="ExternalInput")
        wco_d = nc.dram_tensor("wco", [C, C], BF16, kind="ExternalInput")
        wck_d = nc.dram_tensor("wck", [C, HK], BF16, kind="ExternalInput")
        bq_d = nc.dram_tensor("bq", [P, NCT], F32, kind="ExternalInput")
        bco_d = nc.dram_tensor("bco", [P, NCT], F32, kind="ExternalInput")
        bsep_d = nc.dram_tensor("bsep", [P, NCT], F32, kind="ExternalInput")
        bck_d = nc.dram_tensor("bck", [HK, 1], F32, kind="ExternalInput")
        out_d = nc.dram_tensor("out", [C, L], BF16, kind="ExternalOutput")
        expT_dram = nc.dram_tensor("expTd", [HK, L], BF16)
        recipT_dram = nc.dram_tensor("recipTd", [H, L], BF16)

        # ---- persistent weights / constants ----
        wq8 = prs.tile([P, 6 * C], FP8, tag="wq8", name="wq8")
        pw8 = prs.tile([P, 6 * C], FP8, tag="pw8", name="pw8")
        dg8 = prs.tile([P, NCT * K * P], FP8, tag="dg8", name="dg8")
        nc.sync.dma_start(wq8[:], wq8_d[:])
        nc.sync.dma_start(pw8[:], pw8_d[:])
        nc.sync.dma_start(dg8[:], dg8_d[:])
        wco = [wcop.tile([P, C], BF16, tag="wco", name=f"wco{g}") for g in range(NCT)]
        wck = [wckp.tile([P, HK], BF16, tag="wck", name=f"wck{g}") for g in range(NCT)]
        for g in range(NCT):
            sl = slice(g * P, (g + 1) * P)
            nc.sync.dma_start(wco[g][:], wco_d[sl, :])
            nc.sync.dma_start(wck[g][:], wck_d[sl, :])
        bq = prs.tile([P, NCT], F32, tag="bq", name="bq")
        bco = prs.tile([P, NCT], F32, tag="bco", name="bco")
        bsep = prs.tile([P, NCT], F32, tag="bsep", name="bsep")
        bck = prs.tile([HK, 1], F32, tag="bck", name="bck")
        nc.sync.dma_start(bq[:], bq_d[:])
        nc.sync.dma_start(bco[:], bco_d[:])
        nc.sync.dma_start(bsep[:], bsep_d[:])
        nc.sync.dma_start(bck[:], bck_d[:])
        # ones_block[hk, h] = 1 iff hk // 9 == h (sums exp over k)
        ones = prs.tile([HK, H], BF16, tag="ones", name="ones")
        nc.gpsimd.memset(ones[:], 1.0)
        nc.gpsimd.affine_select(
            out=ones[:], in_=ones[:], compare_op=OP.is_ge, fill=0.0,
            base=0, pattern=[[-K, H]], channel_multiplier=1)
        nc.gpsimd.affine_select(
            out=ones[:], in_=ones[:], compare_op=OP.is_ge, fill=0.0,
            base=K - 1, pattern=[[K, H]], channel_multiplier=-1)

        # ---- persistent activations ----
        # x8: per ctile TWO fp8 rows: row 2g = x[l-4], row 2g+1 = x[l-3]
        # (duplicated shifted copy so conv tap pairs are DoubleRow-able with
        # plane stride XROW %16==0)
        x8 = prs.tile([P, 2 * NCT * XROW], FP8, tag="x8", name="x8")
        dw8 = prs.tile([P, NCT * L], FP8, tag="dw8", name="dw8")  # dwout * SD
        co = [cop.tile([P, CROW], BF16, tag="co", name=f"co{g}") for g in range(NCT)]
        expT = prs.tile([HK, L], BF16, tag="expT", name="expT")
        recipT = prs.tile([H, L], BF16, tag="recipT", name="recipT")
        for r in range(2 * NCT):
            nc.gpsimd.memset(
                bass.AP(x8.tensor, x8.offset + r * XROW,
                        [[2 * NCT * XROW, P], [1, PAD]]), 0.0)
            tail = PAD + L - (r % 2)
            nc.gpsimd.memset(
                bass.AP(x8.tensor, x8.offset + r * XROW + tail,
                        [[2 * NCT * XROW, P], [1, XROW - tail]]), 0.0)
        for g in range(NCT):
            nc.gpsimd.memset(co[g][:, 0:PAD], 0.0)
            nc.gpsimd.memset(co[g][:, PAD + L:CROW], 0.0)

        xap = [2 * NCT * XROW, P]

        def x8ap(row, off, dims):
            return bass.AP(x8.tensor, x8.offset + row * XROW + off,
                           [list(xap)] + dims)

        live = {}

        # ================= pipeline =================
        # Per-step emission order is tuned so no engine queue-head waits on
        # another engine's queue tail: logits/exp (b) first, 2 einsum units
        # (c), softmax tail + kx (b), conv/key/q (a), attn (a), remaining
        # einsum units (c), co (a) last (not needed for 2 more steps).
        for s in range(NLC + 3):
            if s < NLC:
                cs = slice(s * LC, (s + 1) * LC)
                xt = xtp.tile([P, NCT * LC], BF16, tag="xt", name=f"xt{s}")
                nc.sync.dma_start(
                    bass.AP(xt.tensor, xt.offset, [[NCT * LC, P], [LC, NCT], [1, LC]]),
                    bass.AP(xT_d[:].tensor, s * LC, [[L, P], [P * L, NCT], [1, LC]]))
                live[("x", s)] = xt
                nc.sync.dma_start(
                    x8ap(0, PAD + s * LC, [[2 * XROW, NCT], [1, LC]]),
                    bass.AP(x8_d[:].tensor, s * LC, [[L, P], [P * L, NCT], [1, LC]]))
                w = LC if s < NLC - 1 else LC - 1
                nc.sync.dma_start(
                    x8ap(1, PAD + s * LC, [[2 * XROW, NCT], [1, w]]),
                    bass.AP(x8_d[:].tensor, s * LC + 1, [[L, P], [P * L, NCT], [1, w]]))

            a, b, c = s - 1, s - 2, s - 3
            bsl = slice(b * LC, (b + 1) * LC)

            # B1: logits + exp
            if 0 <= b < NLC:
                at = live[("q", b)]
                ps_l = psl.tile([HK, LC], F32, tag="psl", name="pslg")
                for g in range(NCT):
                    nc.tensor.matmul(ps_l[:], wck[g][:], at[:, g * LC:(g + 1) * LC],
                                     start=(g == 0), stop=(g == NCT - 1))
                nc.scalar.activation(expT[:, bsl], ps_l[:], AF.Exp, bias=bck[:, 0:1])

            # C part 1: einsum units g=0,1
            def einsum_unit(g, c, ot):
                kx = live.pop(("kx", g, c))
                base = co[g][:]
                win = bass.AP(base.tensor, base.offset + c * LC,
                              [list(base.ap)[0], [1, K], [1, LC]])
                e = nc.gpsimd if (g, c) in POOL_UNITS else nc.vector
                e.tensor_mul(kx[:], win, kx[:])
                e.tensor_add(kx[:, 0:4, :], kx[:, 0:4, :], kx[:, 4:8, :])
                e.tensor_add(kx[:, 0:2, :], kx[:, 0:2, :], kx[:, 2:4, :])
                e.tensor_add(kx[:, 0, :], kx[:, 0, :], kx[:, 1, :])
                e.tensor_add(ot[:, g * LC:(g + 1) * LC], kx[:, 0, :], kx[:, 8, :])

            otile = None
            if 0 <= c < NLC:
                otile = outp.tile([P, NCT * LC], BF16, tag="o", name=f"o{c}")
                for g in (0, 1):
                    einsum_unit(g, c, otile)

            # B2: sums, recip, recip9 fold, staging, kx broadcast
            if 0 <= b < NLC:
                ps2 = pss.tile([H, LC], F32, tag="pss", name="psss")
                nc.tensor.matmul(ps2[:], ones[:], expT[:, bsl], start=True, stop=True)
                with nc.allow_low_precision(reason="bf16 softmax denominators"):
                    nc.vector.reciprocal(recipT[:, bsl], ps2[:])
                nc.scalar.dma_start(recipT_dram[:, bsl], recipT[:, bsl])
                r9 = r9p.tile([HK, LC], BF16, tag="r9", name=f"r9_{b}")
                rb = recipT_dram[:]
                nc.scalar.dma_start(
                    r9[:], bass.AP(rb.tensor, b * LC, [[L, H], [0, K], [1, LC]]))
                nc.vector.tensor_mul(expT[:, bsl], expT[:, bsl], r9[:])
                nc.scalar.dma_start(expT_dram[:, bsl], expT[:, bsl])
                eb = expT_dram[:]
                for g in range(NCT):
                    kx = kxp.tile([P, K, LC], BF16, tag="kx", name=f"kx{g}_{b}")
                    for hh in range(2):
                        sap = bass.AP(eb.tensor, K * (2 * g + hh) * L + b * LC,
                                      [[0, 64], [L, K], [1, LC]])
                        nc.sync.dma_start(kx[hh * 64:(hh + 1) * 64, :, :], sap)
                    live[("kx", g, b)] = kx

            # A: conv -> key -> q (co deferred)
            if 0 <= a < NLC:
                xt = live[("x", a)]
                qt = qp.tile([P, NCT * LC], BF16, tag="q", name=f"q{a}")
                kt = kp.tile([P, NCT * LC], BF16, tag="k", name=f"k{a}")
                live[("q", a)] = qt
                for g in range(NCT):
                    ps = psb.tile([P, LC], F32, tag="ps", name="psd")
                    for j in range(4):
                        lhsT = bass.AP(dg8.tensor, dg8.offset + (g * K + 2 * j) * P,
                                       [list(dg8.ap[0]), [P, 2], [1, P]])
                        rhs = x8ap(2 * g, a * LC + 2 * j, [[XROW, 2], [1, LC]])
                        nc.tensor.matmul(ps[:], lhsT, rhs, start=(j == 0),
                                         stop=False, perf_mode=DR)
                    lhsT = bass.AP(dg8.tensor, dg8.offset + (g * K + 8) * P,
                                   [list(dg8.ap[0]), [1, P]])
                    rhs = x8ap(2 * g, a * LC + 8, [[1, LC]])
                    nc.tensor.matmul(ps[:], lhsT, rhs, start=False, stop=True)
                    nc.scalar.activation(
                        bass.AP(dw8.tensor, dw8.offset + g * L + a * LC,
                                [list(dw8.ap[0]), [1, LC]]),
                        ps[:], AF.Copy, scale=SD / SW)
                for ot_ in range(NCT):
                    ps = psb.tile([P, LC], F32, tag="ps", name="psk")
                    for j in range(3):
                        lhsT = bass.AP(pw8.tensor, pw8.offset + j * 2 * C + ot_ * P,
                                       [list(pw8.ap[0]), [C, 2], [1, P]])
                        rhs = bass.AP(dw8.tensor, dw8.offset + 2 * j * L + a * LC,
                                      [list(dw8.ap[0]), [L, 2], [1, LC]])
                        nc.tensor.matmul(ps[:], lhsT, rhs, start=(j == 0),
                                         stop=(j == 2), perf_mode=DR)
                    nc.scalar.activation(kt[:, ot_ * LC:(ot_ + 1) * LC], ps[:],
                                         AF.Identity, bias=bsep[:, ot_:ot_ + 1],
                                         scale=1.0 / (SW * SD))
                for ot_ in range(NCT):
                    ps = psb.tile([P, LC], F32, tag="ps", name="psq")
                    for j in range(3):
                        lhsT = bass.AP(wq8.tensor, wq8.offset + j * 2 * C + ot_ * P,
                                       [list(wq8.ap[0]), [C, 2], [1, P]])
                        rhs = x8ap(4 * j, PAD + a * LC, [[2 * XROW, 2], [1, LC]])
                        nc.tensor.matmul(ps[:], lhsT, rhs, start=(j == 0),
                                         stop=(j == 2), perf_mode=DR)
                    nc.scalar.activation(qt[:, ot_ * LC:(ot_ + 1) * LC], ps[:],
                                         AF.Identity, bias=bq[:, ot_:ot_ + 1],
                                         scale=1.0 / SW)
                # attn (per-ot so each fires as soon as its evacs land)
                for ot_ in range(NCT):
                    sl_ = slice(ot_ * LC, (ot_ + 1) * LC)
                    nc.vector.tensor_mul(qt[:, sl_], qt[:, sl_], kt[:, sl_])

            # C part 2 + out store
            if 0 <= c < NLC:
                for g in range(2, NCT):
                    einsum_unit(g, c, otile)
                nc.sync.dma_start(
                    bass.AP(out_d[:].tensor, c * LC, [[L, P], [P * L, NCT], [1, LC]]),
                    bass.AP(otile.tensor, otile.offset,
                            [[NCT * LC, P], [LC, NCT], [1, LC]]))

            # A tail: co (bf16) — consumed by einsum two steps later
            if 0 <= a < NLC:
                xt = live[("x", a)]
                for ot_ in range(NCT):
                    ps = psb.tile([P, LC], F32, tag="ps", name="psc")
                    for g in range(NCT):
                        nc.tensor.matmul(
                            ps[:], wco[g][:, ot_ * P:(ot_ + 1) * P],
                            xt[:, g * LC:(g + 1) * LC],
                            start=(g == 0), stop=(g == NCT - 1))
                    nc.scalar.activation(co[ot_][:, PAD + a * LC:PAD + (a + 1) * LC],
                                         ps[:], AF.Identity, bias=bco[:, ot_:ot_ + 1])


_NC_CACHE = None


def _build():
    global _NC_CACHE
    if _NC_CACHE is None:
        nc = bacc.Bacc("TRN2", target_bir_lowering=False, debug=False)
        with tile.TileContext(nc) as tc:
            _emit(nc, tc)
        nc.compile()
        _NC_CACHE = nc
    return _NC_CACHE


def _host_inputs(hidden_states, W_q, dw, pw, W_ck, W_co, b_q, b_co, sep_bias, b_ck):
    bf = ml_dtypes.bfloat16
    f8 = ml_dtypes.float8_e4m3
    wq8 = np.ascontiguousarray(
        (W_q * SW).reshape(3, 2, P, C).transpose(2, 0, 1, 3).reshape(P, 6 * C)
    ).astype(f8)
    pw8 = np.ascontiguousarray(
        (pw.T * SW).reshape(3, 2, P, C).transpose(2, 0, 1, 3).reshape(P, 6 * C)
    ).astype(f8)
    dws = np.asarray(dw, np.float32).reshape(C, K)
    dg = np.zeros((P, NCT, K, P), np.float32)
    idx = np.arange(P)
    for g in range(NCT):
        for k in range(K):
            dg[idx, g, k, idx] = dws[g * P + idx, k] * SW
    dg8 = np.ascontiguousarray(dg.reshape(P, NCT * K * P)).astype(f8)
    wco = W_co.astype(bf)
    wck = W_ck.astype(bf)
    bq = np.ascontiguousarray(b_q.reshape(NCT, P).T).astype(np.float32)
    bcoh = np.ascontiguousarray(b_co.reshape(NCT, P).T).astype(np.float32)
    bsep = np.ascontiguousarray(sep_bias.reshape(NCT, P).T).astype(np.float32)
    bck = np.asarray(b_ck, np.float32).reshape(HK, 1)
    shared = {"wq8": wq8, "pw8": pw8, "dg8": dg8, "wco": wco, "wck": wck,
              "bq": bq, "bco": bcoh, "bsep": bsep, "bck": bck}
    maps = []
    for b in range(B):
        xT = np.ascontiguousarray(np.asarray(hidden_states[b]).T)
        m = dict(shared)
        m["xT"] = xT.astype(bf)
        m["x8"] = xT.astype(f8)
        maps.append(m)
    return maps


def kernel(hidden_states, W_q, b_q, dw, pw, sep_bias, W_ck, b_ck, W_co, b_co):
    hidden_states = np.asarray(hidden_states, np.float32)
    nc = _build()
    maps = _host_inputs(hidden_states, np.asarray(W_q, np.float32),
                        np.asarray(dw, np.float32), np.asarray(pw, np.float32),
                        np.asarray(W_ck, np.float32), np.asarray(W_co, np.float32),
                        np.asarray(b_q, np.float32), np.asarray(b_co, np.float32),
                        np.asarray(sep_bias, np.float32), np.asarray(b_ck, np.float32))
    res = run_bass_kernel_spmd(nc, maps, list(range(B)))
    out = np.empty((B, L, C), np.float32)
    for b in range(B):
        out[b] = np.asarray(res.results[b]["out"]).T.astype(np.float32)
    return out
